# revision 1
# baseline (speedup 1.0000x reference)
"""Trainium2 Bass kernel for a DiT-style decoder block (adaLN modulation +
attention with QK-RMSNorm + MLP).  B=2, L=2048, D=768, H=12, FF=3072, fp32.

Sharding: 8 cores = 2 batches x 4 sequence-quarters.  Each core computes
adaLN(s_att/sh_att) + LayerNorm + K/V for its batch's full 2048 tokens
(4x replicated work, no collectives) and attention / to_out / MLP for its
own 512 query tokens only.

v3: bf16 data throughout (halves HBM traffic, enables DVE 2x/4x fast
modes); fp8e4m3 + DoubleRow for A@V and the W2 GEMM (2x PE rate); stages
A/A2 fused per token-group (no DRAM h roundtrip); PSUM+bias reads routed
through scalar-engine Identity activations so DVE elementwise ops run on
SBUF bf16 at 4x; softmax exp runs on [128,1024] double-PSUM-bank tiles;
the adaLN g_att/s_mlp/sh_mlp/g_mlp matmuls (they only need silu(c)) are
interleaved into the attention stage where the PE is otherwise idle while
the scalar engine streams exp; Wo/W2 weights prefetch during attention.
The residual path (x, x2, output) stays fp32.  Per-token reductions are
ones-vector matmuls on the PE; per-token rows are broadcast via K=1
matmuls and landed in SBUF bf16.  Softmax denominators come free from the
A@V matmul via a 65-column-per-head V layout ([v_h | 1]); exp(sim-2)
keeps es inside fp8e4m3 range.  The V projection bias is folded into an
effective to_out bias on the host (softmax rows sum to 1, so it commutes
through attention).
"""

import sys

if '/opt/trn_rl_repo' not in sys.path:
    sys.path.insert(0, '/opt/trn_rl_repo')

import contextlib
import os

import numpy as np
import ml_dtypes

import concourse.bass as bass
import concourse.mybir as mybir
import concourse.tile as tile
from concourse import bacc
from concourse.bass_utils import run_bass_kernel_spmd

F32 = mybir.dt.float32
F32R = mybir.dt.float32r
BF16 = mybir.dt.bfloat16
FP8 = mybir.dt.float8e4
AF = mybir.ActivationFunctionType
OP = mybir.AluOpType
DR = mybir.MatmulPerfMode.DoubleRow

NPBF = ml_dtypes.bfloat16
NPF8 = ml_dtypes.float8_e4m3

B, L, D, HEADS, DH, FF = 2, 2048, 768, 12, 64, 3072
P = 128
DC = D // P              # 6
FFC = FF // P            # 24
KC = L // P              # 16 key chunks
N_CORES = 8
QT = L // 4              # 512
NG = L // 512            # 4 token groups
SQRT_D = float(np.sqrt(D))
SCALE = DH ** -0.5
LN_EPS = 1e-6
ESH = 2.0                # exp shift: es = exp(sim - ESH) stays in fp8 range
W2_PRE = 16.0            # fp8 W2 host prescale (keeps weights in e4m3 normal range)
STAGE = int(os.environ.get('BASS_STAGE_LIMIT', '6'))
SIM_COMPAT = bool(os.environ.get("BASS_SIM_COMPAT"))

PACKS = ("one_p_bada_s", "bada_sh", "bada_ga", "one_p_bada_sm", "bada_shm",
         "bada_gm16", "bk", "bq", "bo_eff", "go_s", "gq_s", "gk_s", "b2")


def r32(ap):
    return ap.bitcast(F32R)


def _emit(nc):
    din = {}

    def dram_in(name, shape, dt=F32):
        din[name] = nc.dram_tensor(name, shape, dt, kind="ExternalInput").ap()
        return din[name]

    xT = dram_in("xT", [D, L], BF16)
    cT = dram_in("cT", [D, L], BF16)
    xT_own = dram_in("xT_own", [D, QT])       # clean f32 copy for the residual
    Wada = dram_in("Wada", [D, 6 * D], BF16)
    Wqkv = dram_in("Wqkv", [D, 3 * D], BF16)
    Wo = dram_in("Wo", [D, D], BF16)
    W1 = dram_in("W1", [D, FF], BF16)
    W2 = dram_in("W2", [FF, D], FP8)
    ONES = dram_in("ONES", [P, 512])
    for name in PACKS:
        dram_in(name, [P, DC])
    dram_in("b1", [P, FFC])

    outT = nc.dram_tensor("outT", [D, QT], F32, kind="ExternalOutput").ap()

    xT_c = xT.rearrange("(j p) t -> p j t", p=P)
    cT_c = cT.rearrange("(j p) t -> p j t", p=P)
    xTo_c = xT_own.rearrange("(j p) t -> p j t", p=P)
    Wada_c = Wada.rearrange("(k p) n -> p k n", p=P)
    Wqkv_c = Wqkv.rearrange("(k p) n -> p k n", p=P)
    Wo_c = Wo.rearrange("(k p) n -> p k n", p=P)
    W1_c = W1.rearrange("(k p) n -> p k n", p=P)
    W2_c = W2.rearrange("(f two p) n -> p f two n", p=P, two=2)
    outT_c = outT.rearrange("(j p) t -> p j t", p=P)

    with tile.TileContext(nc) as tc, contextlib.ExitStack() as ctx:
        const = ctx.enter_context(tc.tile_pool(name="const", bufs=1))
        stat = ctx.enter_context(tc.tile_pool(name="stat", bufs=1))
        rows = ctx.enter_context(tc.tile_pool(name="rows", bufs=2))

        ones = const.tile([P, 512], F32, tag="ones")
        nc.sync.dma_start(r32(ones[:]), r32(ONES))
        onesb = const.tile([P, 512], BF16, tag="onesb")
        nc.vector.tensor_copy(onesb[:], ones[:])
        onescb = onesb[:, 0:1]        # bf16 column (for bf16 stat sums)
        onesc = ones[:, 0:1]          # f32 column (for f32r stat sums)

        pk = {}
        for name in PACKS:
            t = const.tile([P, DC], F32, tag=name)
            nc.sync.dma_start(t[:], din[name])
            pk[name] = t
        b1_pp = const.tile([P, FFC], F32, tag="b1")
        nc.sync.dma_start(b1_pp[:], din["b1"])

        sc_own = const.tile([P, DC, QT], BF16, tag="sc_own")
        x_own = const.tile([P, DC, QT], F32, tag="x_own")
        nc.sync.dma_start(x_own[:], xTo_c)

        def stat_sum_bf(dst_ps, src_bf, start, stop):
            nc.tensor.matmul(dst_ps, onescb, src_bf, start=start, stop=stop)

        def stat_sum_f32(dst_ps, src_f32, start, stop):
            nc.tensor.matmul(dst_ps, r32(onesc), r32(src_f32), start=start,
                             stop=stop)

        def publish(row_f32):
            """per-token row -> bf16 so the bcast matmul runs bf16."""
            t = rows.tile([1, 512], BF16, tag="pub")
            nc.vector.tensor_copy(t[:], row_f32[:])
            return t

        def bcast_row(dst_ps, row_bf, m=P):
            nc.tensor.matmul(dst_ps, onesb[0:1, 0:m], row_bf, start=True, stop=True)

        def recip_rows(src_ps, clamp):
            """rows = 1/max(sqrt(ps), clamp), published to bf16."""
            a_row = stat.tile([1, 512], F32, tag="srowA")
            b_row = stat.tile([1, 512], F32, tag="srowB")
            nc.scalar.activation(a_row[:], src_ps, AF.Sqrt)
            if clamp is not None:
                nc.vector.tensor_scalar_max(a_row[:], a_row[:], clamp)
            nc.vector.reciprocal(b_row[:], a_row[:])
            return publish(b_row)

        def ln_rows(ps_s, ps_ss, psb_pool):
            """mean/var -> (r_bcast, -mean*r_bcast) PSUM broadcasts."""
            m_row = stat.tile([1, 512], F32, tag="srowC")
            v_row = stat.tile([1, 512], F32, tag="srowD")
            t_row = stat.tile([1, 512], F32, tag="srowE")
            r_row = stat.tile([1, 512], F32, tag="srowF")
            nc.scalar.activation(m_row[:], ps_s, AF.Copy, scale=1.0 / D)
            nc.scalar.activation(v_row[:], ps_ss, AF.Copy, scale=1.0 / D)
            nc.vector.tensor_tensor(t_row[:], m_row[:], m_row[:], OP.mult)
            nc.vector.tensor_sub(v_row[:], v_row[:], t_row[:])
            nc.scalar.activation(v_row[:], v_row[:], AF.Sqrt, bias=LN_EPS)
            nc.vector.reciprocal(r_row[:], v_row[:])
            nc.vector.scalar_tensor_tensor(t_row[:], m_row[:], -1.0, r_row[:],
                                           OP.mult, OP.mult)
            rp = publish(r_row)
            mp = publish(t_row)
            ps_r = psb_pool.tile([P, 512], F32, tag="ps_r")
            ps_m = psb_pool.tile([P, 512], F32, tag="ps_mrB")
            bcast_row(ps_r[:], rp[:])
            bcast_row(ps_m[:], mp[:])
            return ps_r, ps_m

        # ======= Stage A+A2 fused: per token-group: silu(c), LN1, mod1, h,
        # then K/V (and Q for the own group) straight from SBUF h =======
        kT = const.tile([P, DC, L], BF16, tag="kT")
        v65 = const.tile([P, KC, HEADS * 65 + 4], FP8, tag="v65")  # pad row to 784 (16B-aligned kc stride for DoubleRow)
        qT = const.tile([P, DC, QT], BF16, tag="qT")

        with tc.tile_pool(name="pA", bufs=2) as pA, \
             tc.tile_pool(name="pA1", bufs=1) as pA1, \
             tc.tile_pool(name="psA", bufs=2, space="PSUM") as psA, \
             tc.tile_pool(name="psAs", bufs=1, space="PSUM") as psAs, \
             tc.tile_pool(name="psAb", bufs=1, space="PSUM") as psAb:
            wada1 = pA1.tile([P, DC, 2 * D], BF16, tag="wada1")
            nc.sync.dma_start(wada1[:], Wada_c[:, :, 0:2 * D])
            wqkv = pA1.tile([P, DC, 3 * D], BF16, tag="wqkv")
            nc.sync.dma_start(wqkv[:], Wqkv_c[:])
            # fp8 ones into the 65th column of each head's V block
            for kc in range(KC):
                vt = v65[:, kc, :]
                dst = bass.AP(vt.tensor, vt.offset + DH,
                              [list(vt.ap[0]), [65, HEADS]])
                nc.vector.tensor_copy(dst, ones[:, 0:HEADS])
            for g in range(NG):
                gs = slice(g * 512, (g + 1) * 512)
                c_g = pA.tile([P, DC, 512], BF16, tag="c_g")
                x_g = pA.tile([P, DC, 512], BF16, tag="x_g")
                nc.sync.dma_start(c_g[:], cT_c[:, :, gs])
                nc.sync.dma_start(x_g[:], xT_c[:, :, gs])
                sc_g = pA1.tile([P, DC, 512], BF16, tag="sc_g")
                if SIM_COMPAT:
                    nc.scalar.activation(sc_g[:], c_g[:], AF.Sigmoid)
                    nc.vector.tensor_tensor(sc_g[:], sc_g[:], c_g[:], OP.mult)
                else:
                    nc.scalar.activation(sc_g[:], c_g[:], AF.Silu)
                if g == 0:
                    nc.vector.tensor_copy(sc_own[:], sc_g[:])
                # --- LN1 stats ---
                ps_s = psAs.tile([1, 512], F32, tag="st_a")
                ps_ss = psAs.tile([1, 512], F32, tag="st_b")
                for j in range(DC):
                    stat_sum_bf(ps_s[:], x_g[:, j, :], j == 0, j == DC - 1)
                for j in range(DC):
                    xsq = pA.tile([P, 512], BF16, tag="xsq")
                    nc.vector.tensor_tensor(xsq[:], x_g[:, j, :], x_g[:, j, :],
                                            OP.mult)
                    stat_sum_bf(ps_ss[:], xsq[:], j == 0, j == DC - 1)
                ps_r, ps_m = ln_rows(ps_s[:], ps_ss[:], psAb)
                r_sb = pA.tile([P, 512], BF16, tag="r_sb")
                m_sb = pA.tile([P, 512], BF16, tag="m_sb")
                nc.scalar.activation(r_sb[:], ps_r[:], AF.Copy)
                nc.scalar.activation(m_sb[:], ps_m[:], AF.Copy)
                # --- mod1 + h (bf16) ---
                xh_g = pA1.tile([P, DC, 512], BF16, tag="xh_g")
                for j in range(DC):
                    nc.vector.tensor_tensor(xh_g[:, j, :], x_g[:, j, :], r_sb[:],
                                            OP.mult)
                    nc.vector.tensor_tensor(xh_g[:, j, :], xh_g[:, j, :],
                                            m_sb[:], OP.add)
                h_g = pA1.tile([P, DC, 512], BF16, tag="h_g")
                for j in range(DC):
                    ps_sj = psA.tile([P, 512], F32, tag="ps_sj")
                    ps_shj = psA.tile([P, 512], F32, tag="ps_shj")
                    for ki in range(DC):
                        nc.tensor.matmul(ps_sj[:],
                                         wada1[:, ki, j * P:(j + 1) * P],
                                         sc_g[:, ki, :],
                                         start=(ki == 0), stop=(ki == DC - 1))
                    for ki in range(DC):
                        nc.tensor.matmul(ps_shj[:],
                                         wada1[:, ki, D + j * P:D + (j + 1) * P],
                                         sc_g[:, ki, :],
                                         start=(ki == 0), stop=(ki == DC - 1))
                    t1 = pA.tile([P, 512], BF16, tag="t1")
                    t2 = pA.tile([P, 512], BF16, tag="t2")
                    nc.scalar.activation(t1[:], ps_sj[:], AF.Identity,
                                         bias=pk["one_p_bada_s"][:, j:j + 1])
                    nc.scalar.activation(t2[:], ps_shj[:], AF.Identity,
                                         bias=pk["bada_sh"][:, j:j + 1])
                    tmp = pA.tile([P, 512], BF16, tag="hfold")
                    nc.vector.tensor_tensor(tmp[:], t1[:], xh_g[:, j, :], OP.mult)
                    nc.vector.tensor_tensor(h_g[:, j, :], tmp[:], t2[:], OP.add)
                if STAGE < 2:
                    continue
                # --- k (for this group's tokens) ---
                ps_kss = psAs.tile([1, 512], F32, tag="st_a")
                for j in range(DC):
                    ps_k = psA.tile([P, 512], F32, tag="ps_sj")
                    for ki in range(DC):
                        nc.tensor.matmul(ps_k[:],
                                         wqkv[:, ki, D + j * P:D + (j + 1) * P],
                                         h_g[:, ki, :],
                                         start=(ki == 0), stop=(ki == DC - 1))
                    nc.scalar.activation(kT[:, j, gs], ps_k[:], AF.Identity,
                                         bias=pk["bk"][:, j:j + 1])
                    ksq = pA.tile([P, 512], BF16, tag="ksq")
                    nc.vector.tensor_tensor(ksq[:], kT[:, j, gs], kT[:, j, gs],
                                            OP.mult)
                    stat_sum_bf(ps_kss[:], ksq[:], j == 0, j == DC - 1)
                rkp = recip_rows(ps_kss[:], 1e-12)
                ps_rk = psAb.tile([P, 512], F32, tag="ps_r")
                bcast_row(ps_rk[:], rkp[:])
                rk_sb = pA.tile([P, 512], BF16, tag="rk_sb")
                nc.scalar.activation(rk_sb[:], ps_rk[:], AF.Copy)
                for j in range(DC):
                    nc.vector.scalar_tensor_tensor(
                        kT[:, j, gs], kT[:, j, gs], pk["gk_s"][:, j:j + 1],
                        rk_sb[:], OP.mult, OP.mult)
                # --- v (token-major, 65 cols/head: [v_h | 1], fp8) ---
                for tt in range(g * 4, (g + 1) * 4):
                    lt = tt % 4
                    for n in range(2):
                        ps_vt = psA.tile([P, 512], F32, tag="ps_shj")
                        ps_v = ps_vt[:, 0:384]
                        for ki in range(DC):
                            nc.tensor.matmul(
                                ps_v, h_g[:, ki, lt * P:(lt + 1) * P],
                                wqkv[:, ki, 2 * D + n * 384:2 * D + (n + 1) * 384],
                                start=(ki == 0), stop=(ki == DC - 1))
                        vt = v65[:, tt, :]
                        dst = bass.AP(vt.tensor, vt.offset + n * 6 * 65,
                                      [list(vt.ap[0]), [65, 6], [1, DH]])
                        nc.scalar.activation(
                            dst, ps_v.rearrange("p (h d) -> p h d", h=6), AF.Copy)
                # --- q (own tokens only) ---
                if g == 0:
                    ps_qss = psAs.tile([1, 512], F32, tag="st_b")
                    for j in range(DC):
                        ps_q = psA.tile([P, 512], F32, tag="ps_sj")
                        for ki in range(DC):
                            nc.tensor.matmul(ps_q[:],
                                             wqkv[:, ki, j * P:(j + 1) * P],
                                             h_g[:, ki, :],
                                             start=(ki == 0), stop=(ki == DC - 1))
                        nc.scalar.activation(qT[:, j, :], ps_q[:], AF.Identity,
                                             bias=pk["bq"][:, j:j + 1])
                        qsq = pA.tile([P, 512], BF16, tag="ksq")
                        nc.vector.tensor_tensor(qsq[:], qT[:, j, :], qT[:, j, :],
                                                OP.mult)
                        stat_sum_bf(ps_qss[:], qsq[:], j == 0, j == DC - 1)
                    rqp = recip_rows(ps_qss[:], 1e-12)
                    ps_rq = psAb.tile([P, 512], F32, tag="ps_mrB")
                    bcast_row(ps_rq[:], rqp[:])
                    rq_sb = pA.tile([P, 512], BF16, tag="rk_sb")
                    nc.scalar.activation(rq_sb[:], ps_rq[:], AF.Copy)
                    for j in range(DC):
                        nc.vector.scalar_tensor_tensor(
                            qT[:, j, :], qT[:, j, :], pk["gq_s"][:, j:j + 1],
                            rq_sb[:], OP.mult, OP.mult)

        # precomputed adaLN tiles (filled during stage B, used by C/D/E);
        # allocated after stage A's pools free their SBUF
        mid = ctx.enter_context(tc.tile_pool(name="mid", bufs=1))
        ga_t = mid.tile([P, DC, QT], BF16, tag="ga_t")
        sm_t = mid.tile([P, DC, QT], BF16, tag="sm_t")
        shm_t = mid.tile([P, DC, QT], BF16, tag="shm_t")
        gm_t = mid.tile([P, DC, QT], BF16, tag="gm_t")
        wo_sb = mid.tile([P, DC, D], BF16, tag="wo_sb")
        w2_sb = mid.tile([P, FFC // 2, 2, D], FP8, tag="w2_sb")
        w1_sb = mid.tile([P, DC, FF], BF16, tag="w1_sb")

        if STAGE >= 3:
            # =========== Stage B: attention (+ interleaved adaLN mod2) =====
            # sim (bf16) -> [128,1024] PSUM pairs; one exp per pair -> es8;
            # A@V in fp8 DoubleRow, denominator via the ones column of v65.
            # The PE is mostly idle while Act streams exp, so the
            # sc_own-only adaLN matmuls (ga/sm/shm/gm) and the Wo/W2
            # prefetches are interleaved between heads.
            attn = mid.tile([P, DC, QT], BF16, tag="attn")

            def pre_block(bi, pB, psPre):
                kind, j = divmod(bi, DC)
                name, dst, boff = [
                    ("ga", ga_t, 2 * D), ("sm", sm_t, 3 * D),
                    ("shm", shm_t, 4 * D), ("gm", gm_t, 5 * D)][kind]
                wt = pB.tile([P, DC, P], BF16, tag="wpre")
                nc.sync.dma_start(wt[:], Wada_c[:, :, boff + j * P:boff + (j + 1) * P])
                ps = psPre.tile([P, 512], F32, tag="ps_pre")
                for ki in range(DC):
                    nc.tensor.matmul(ps[:], wt[:, ki, :], sc_own[:, ki, :],
                                     start=(ki == 0), stop=(ki == DC - 1))
                bias = {"ga": "bada_ga", "sm": "one_p_bada_sm",
                        "shm": "bada_shm", "gm": "bada_gm16"}[name]
                if name == "gm":
                    nc.vector.tensor_scalar(dst[:, j, :], ps[:],
                                            pk[bias][:, j:j + 1],
                                            1.0 / W2_PRE, OP.add, OP.mult)
                else:
                    nc.vector.tensor_scalar_add(dst[:, j, :], ps[:],
                                                pk[bias][:, j:j + 1])

            with tc.tile_pool(name="pB", bufs=2) as pB, \
                 tc.tile_pool(name="psB", bufs=2, space="PSUM") as psB, \
                 tc.tile_pool(name="psBa", bufs=1, space="PSUM") as psBa, \
                 tc.tile_pool(name="psBb", bufs=1, space="PSUM") as psBb, \
                 tc.tile_pool(name="psPre", bufs=1, space="PSUM") as psPre:
                nc.sync.dma_start(wo_sb[:], Wo_c[:])
                nc.sync.dma_start(w2_sb[:], W2_c[:])
                for h in range(HEADS):
                    po = (h % 2) * DH
                    ch = h // 2
                    es8 = pB.tile([P, KC // 2, 2, 512], FP8, tag="es8")
                    ps_av = psBa.tile([65, 512], F32, tag="ps_av")
                    for i in range(KC // 2):
                        ps_sim = psB.tile([P, 1024], F32, tag="ps_sim")
                        for half in range(2):
                            kc = 2 * i + half
                            ks = slice(kc * P, (kc + 1) * P)
                            nc.tensor.matmul(
                                ps_sim[:, half * 512:(half + 1) * 512],
                                kT[po:po + DH, ch, ks],
                                qT[po:po + DH, ch, :],
                                start=True, stop=True)
                        nc.scalar.activation(es8[:, i, :, :], ps_sim[:],
                                             AF.Exp, bias=-ESH)
                        nc.tensor.matmul(ps_av[:],
                                         v65[:, 2 * i:2 * i + 2,
                                             h * 65:(h + 1) * 65],
                                         es8[:, i, :, :],
                                         start=(i == 0), stop=(i == KC // 2 - 1),
                                         perf_mode=DR)
                    se_row = stat.tile([1, 512], F32, tag="srowA")
                    rec_row = stat.tile([1, 512], F32, tag="srowB")
                    nc.vector.tensor_copy(se_row[:], ps_av[DH:DH + 1, :])
                    nc.vector.reciprocal(rec_row[:], se_row[:])
                    rcp = publish(rec_row)
                    ps_rec = psBb.tile([DH, 512], F32, tag="ps_rec")
                    bcast_row(ps_rec[:], rcp[:], m=DH)
                    rec_b = pB.tile([DH, 512], BF16, tag="rec_b")
                    nc.vector.tensor_copy(rec_b[:], ps_rec[:])
                    nc.vector.tensor_tensor(attn[po:po + DH, ch, :],
                                            ps_av[0:DH, :], rec_b[:], OP.mult)
                    # two adaLN precompute blocks per head
                    for bi in (2 * h, 2 * h + 1):
                        pre_block(bi, pB, psPre)

        if STAGE >= 4:
            # =========== Stage C: to_out + rmsnorm_o + residual ===========
            late = ctx.enter_context(tc.tile_pool(name="late", bufs=1))
            x2 = late.tile([P, DC, QT], F32, tag="x2")
            h2 = late.tile([P, DC, QT], BF16, tag="h2")
            nc.sync.dma_start(w1_sb[:], W1_c[:])
            with tc.tile_pool(name="pC", bufs=2) as pC, \
                 tc.tile_pool(name="pC1", bufs=1) as pC1, \
                 tc.tile_pool(name="psC", bufs=2, space="PSUM") as psC, \
                 tc.tile_pool(name="psCs", bufs=1, space="PSUM") as psCs, \
                 tc.tile_pool(name="psCb", bufs=1, space="PSUM") as psCb, \
                 tc.tile_pool(name="psCd", bufs=1, space="PSUM") as psCd:
                ps_yss = psCs.tile([1, 512], F32, tag="ps_yss")
                y_sb = pC1.tile([P, DC, 512], BF16, tag="y_sb")
                for j in range(DC):
                    ps_y = psC.tile([P, 512], F32, tag="ps_y")
                    for ki in range(DC):
                        nc.tensor.matmul(ps_y[:], wo_sb[:, ki, j * P:(j + 1) * P],
                                         attn[:, ki, :],
                                         start=(ki == 0), stop=(ki == DC - 1))
                    nc.scalar.activation(y_sb[:, j, :], ps_y[:], AF.Identity,
                                         bias=pk["bo_eff"][:, j:j + 1])
                    ysq = pC.tile([P, 512], BF16, tag="ysq")
                    nc.vector.tensor_tensor(ysq[:], y_sb[:, j, :], y_sb[:, j, :],
                                            OP.mult)
                    stat_sum_bf(ps_yss[:], ysq[:], j == 0, j == DC - 1)
                rop = recip_rows(ps_yss[:], 1e-12)
                ps_ro = psCb.tile([P, 512], F32, tag="ps_ro")
                bcast_row(ps_ro[:], rop[:])
                ro_sb = pC.tile([P, 512], BF16, tag="ro_sb")
                nc.scalar.activation(ro_sb[:], ps_ro[:], AF.Copy)
                ps_s2 = psCd.tile([1, 512], F32, tag="ps_s2")
                ps_ss2 = psCd.tile([1, 512], F32, tag="ps_ss2")
                for j in range(DC):
                    t = pC.tile([P, 512], BF16, tag="cf1")
                    nc.vector.scalar_tensor_tensor(t[:], y_sb[:, j, :],
                                                   pk["go_s"][:, j:j + 1],
                                                   ro_sb[:], OP.mult, OP.mult)
                    t2 = pC.tile([P, 512], BF16, tag="cf2")
                    nc.vector.tensor_tensor(t2[:], ga_t[:, j, :], t[:], OP.mult)
                    nc.vector.tensor_tensor(r32(x2[:, j, :]), x_own[:, j, :],
                                            t2[:], OP.add)
                    # LN2 stats interleaved as soon as each x2 chunk lands
                    stat_sum_f32(ps_s2[:], x2[:, j, :], j == 0, j == DC - 1)
                    x2sq = pC.tile([P, 512], BF16, tag="x2sq")
                    nc.vector.tensor_tensor(x2sq[:], x2[:, j, :], x2[:, j, :],
                                            OP.mult)
                    stat_sum_bf(ps_ss2[:], x2sq[:], j == 0, j == DC - 1)

        if STAGE >= 5:
            # =========== Stage D: LN2 + h2 (stats came from stage C) ======
            with tc.tile_pool(name="pD", bufs=2) as pD, \
                 tc.tile_pool(name="psDb", bufs=1, space="PSUM") as psDb:
                ps_r2, ps_m2 = ln_rows(ps_s2[:], ps_ss2[:], psDb)
                r2_sb = pD.tile([P, 512], BF16, tag="r2_sb")
                m2_sb = pD.tile([P, 512], BF16, tag="m2_sb")
                nc.scalar.activation(r2_sb[:], ps_r2[:], AF.Copy)
                nc.scalar.activation(m2_sb[:], ps_m2[:], AF.Copy)
                for j in range(DC):
                    x2h = pD.tile([P, 512], BF16, tag="x2h")
                    nc.vector.tensor_tensor(x2h[:], x2[:, j, :], r2_sb[:],
                                            OP.mult)
                    nc.vector.tensor_tensor(x2h[:], x2h[:], m2_sb[:], OP.add)
                    tmp = pD.tile([P, 512], BF16, tag="h2fold")
                    nc.vector.tensor_tensor(tmp[:], sm_t[:, j, :], x2h[:],
                                            OP.mult)
                    nc.vector.tensor_tensor(h2[:, j, :], tmp[:], shm_t[:, j, :],
                                            OP.add)

        if STAGE >= 6:
            # =========== Stage E: MLP + g_mlp + output ===========
            with tc.tile_pool(name="pE", bufs=2) as pE, \
                 tc.tile_pool(name="pE1", bufs=1) as pE1, \
                 tc.tile_pool(name="psE", bufs=2, space="PSUM") as psE:
                ms = pE1.tile([P, FFC, 512], FP8, tag="ms")
                for f in range(FFC):
                    ps_m1 = psE.tile([P, 512], F32, tag="ps_m1")
                    for ki in range(DC):
                        nc.tensor.matmul(ps_m1[:],
                                         w1_sb[:, ki, f * P:(f + 1) * P],
                                         h2[:, ki, :],
                                         start=(ki == 0), stop=(ki == DC - 1))
                    if SIM_COMPAT:
                        nc.scalar.activation(ms[:, f, :], ps_m1[:], AF.Sigmoid,
                                             bias=b1_pp[:, f:f + 1])
                        nc.vector.scalar_tensor_tensor(
                            ms[:, f, :], ps_m1[:], b1_pp[:, f:f + 1],
                            ms[:, f, :], OP.add, OP.mult)
                    else:
                        nc.scalar.activation(ms[:, f, :], ps_m1[:], AF.Silu,
                                             bias=b1_pp[:, f:f + 1])
                for j in range(DC):
                    ps_o = psE.tile([P, 512], F32, tag="ps_m1")
                    for f in range(FFC // 2):
                        nc.tensor.matmul(ps_o[:], w2_sb[:, f, :, j * P:(j + 1) * P],
                                         ms[:, 2 * f:2 * f + 2, :],
                                         start=(f == 0), stop=(f == FFC // 2 - 1),
                                         perf_mode=DR)
                    t = pE.tile([P, 512], BF16, tag="efold")
                    nc.vector.scalar_tensor_tensor(t[:], ps_o[:],
                                                   pk["b2"][:, j:j + 1],
                                                   gm_t[:, j, :], OP.add, OP.mult)
                    out_j = pE.tile([P, 512], F32, tag="out_j")
                    nc.vector.tensor_tensor(out_j[:], x2[:, j, :], t[:], OP.add)
                    nc.sync.dma_start(outT_c[:, j, :], out_j[:])
        if STAGE < 4:
            # debug escape: write something so the output tensor is bound
            dbg = const.tile([P, DC, QT], F32, tag="dbg")
            nc.vector.tensor_copy(dbg[:], sc_own[:])
            nc.sync.dma_start(outT_c, dbg[:])


_BUILT = None


def _register_const(nc, value):
    t = nc.alloc_sbuf_tensor(f"const-float32-{value}", [128, 1], F32)
    nc.gpsimd.memset(t.ap(), value)
    nc.const_aps.aps[(F32, value)] = t.ap()
    nc.all_engine_barrier()


def _build():
    global _BUILT
    if _BUILT is None:
        nc = bacc.Bacc("TRN2", target_bir_lowering=False, debug=False,
                       num_devices=N_CORES)
        _register_const(nc, LN_EPS)
        _register_const(nc, -ESH)
        _emit(nc)
        nc.compile()
        _BUILT = nc
    return _BUILT


def _pack_pp(vec, nchunk):
    return np.ascontiguousarray(np.asarray(vec, np.float32).reshape(nchunk, P).T)


def make_in_maps(inputs):
    f = lambda k: np.asarray(inputs[k], dtype=np.float32)
    x, c = f("x"), f("c")
    b_ada, b_qkv, b_o = f("b_ada"), f("b_qkv"), f("b_o")
    bo_eff = b_o + b_qkv[2 * D:] @ f("W_o")   # v-bias folded through attention
    bf = lambda a: np.ascontiguousarray(np.asarray(a, dtype=NPBF))
    shared = {
        "Wada": bf(f("W_ada")), "Wqkv": bf(f("W_qkv")), "Wo": bf(f("W_o")),
        "W1": bf(f("W1")),
        "W2": np.ascontiguousarray(np.asarray(f("W2") * W2_PRE, dtype=NPF8)),
        "ONES": np.ones((P, 512), np.float32),
        "one_p_bada_s": _pack_pp(1.0 + b_ada[:D], DC),
        "bada_sh": _pack_pp(b_ada[D:2 * D], DC),
        "bada_ga": _pack_pp(b_ada[2 * D:3 * D], DC),
        "one_p_bada_sm": _pack_pp(1.0 + b_ada[3 * D:4 * D], DC),
        "bada_shm": _pack_pp(b_ada[4 * D:5 * D], DC),
        "bada_gm16": _pack_pp(b_ada[5 * D:] / W2_PRE, DC),
        "bk": _pack_pp(b_qkv[D:2 * D], DC),
        "bq": _pack_pp(b_qkv[:D], DC),
        "bo_eff": _pack_pp(bo_eff, DC),
        "go_s": _pack_pp(f("g_o") * SQRT_D, DC),
        "gq_s": _pack_pp(f("g_q") * SQRT_D * SCALE, DC),
        "gk_s": _pack_pp(f("g_k") * SQRT_D, DC),
        "b2": _pack_pp(f("b2") * W2_PRE, DC),
        "b1": _pack_pp(f("b1"), FFC),
    }
    in_maps = []
    for core in range(N_CORES):
        b = core // 4
        q0 = (core % 4) * QT
        perm = np.r_[q0:q0 + QT, 0:q0, q0 + QT:L]
        m = dict(shared)
        m["xT"] = bf(x[b][perm].T)
        m["cT"] = bf(c[b][perm].T)
        m["xT_own"] = np.ascontiguousarray(x[b][q0:q0 + QT].T)
        in_maps.append(m)
    return in_maps


def assemble_out(results):
    out = np.empty((B, L, D), dtype=np.float32)
    for core in range(N_CORES):
        b = core // 4
        q0 = (core % 4) * QT
        out[b, q0:q0 + QT] = results[core]["outT"].T
    return out


def kernel(**inputs):
    nc = _build()
    in_maps = make_in_maps(inputs)
    res = run_bass_kernel_spmd(nc, in_maps, core_ids=list(range(N_CORES)))
    return assemble_out(res.results)

